# revision 35
# baseline (speedup 1.0000x reference)
"""Trainium2 Bass kernel for nn_Mlp_StaticRoutedLoRAExpert.

Computation (per token chunk with static expert e):
    h = gelu(x @ w1.T + bias1 + SCALE * (x @ a1[e].T) @ b1[e].T)
    y = h @ w2.T + bias2 + SCALE * (h @ a2[e].T) @ b2[e].T

Since experts are static per chunk, the LoRA factors fold into the dense
weights on the host (exact math, done in float64):
    W1eff[e] = w1 + SCALE * b1[e] @ a1[e]        # [HID, IN]
    W2eff[e] = w2 + SCALE * b2[e] @ a2[e]        # [OUT, HID]
so the device kernel is a plain two-layer MLP with a per-chunk weight
select.  All device tensors are bf16 (1 cycle/row on the PE array, same
as fp32r, half the SBUF/DMA) except PSUM accumulation (f32) and biases.

Sharding: data-parallel over batch, 4 batches per core on 8 cores, no
collectives.  Feature-major layout (X^T, H^T, Y^T): features on
partitions, tokens as the matmul moving dim.  Per 512-token tile:
  fc1 (6x24 matmuls) -> gelu+bias1 on Act -> H^T in SBUF (never DRAM)
  fc2 (24x6 matmuls into 6 PSUM banks) -> bias2+cast drain on Vector
Tokens are regrouped by expert on the host (tiles are just column sets),
so all tiles are a uniform 512 wide: 10 tiles/core, 2880 matmuls instead
of 3456.  x is pre-blocked so each tile is ONE contiguous DMA and y is
stored once per tile PAIR (the last pair per tile, the final store in
o-aligned 3T/2T/T pieces, to shorten the tail); x loads in k-halves; x/y
ride the Act HWDGE ring, weights the SP ring in first-used-expert order
with w1 split in hidden-dim twelfths and a dummy gelu pre-warming the
Act table, so the PE starts ~5us in and never stalls again (sim: one
startup gap, zero mid-kernel gaps, 4.1us tail).  Per core: ~48 DMAs /
~6k descriptors / 35 MB HBM traffic (vs ~730 / ~93k / 187 MB for the
two-phase f32r baseline, whose DRAM H^T roundtrip dominated the measured
span; descriptor counts below ring capacity ~10k preload for free).
TimelineSim 626.0 us/core = 1.72% above the bf16 matmul floor (615.4 us,
288 PE-cycles/token); differential HW probe ~0.75-1.0 ms/iter (baseline
~2.2 ms/iter the same way).
"""

import numpy as np
import ml_dtypes

SCALE = 128.0 / 64.0
B, S, IN, HID, OUT, E, R = 32, 1280, 768, 3072, 768, 2, 64
NCORES = 8
BPC = B // NCORES          # batches per core
TPC = BPC * S              # tokens per core
P = 128
KI = IN // P               # 6  input k-chunks
KH = HID // P              # 24 hidden chunks
KO = OUT // P              # 6  output chunks
MAX_T = 512                # one PSUM bank of f32 per matmul output
W1SPLIT = 12               # w1 loads in hidden-dim twelfths: PE starts sooner
HID4 = HID // W1SPLIT
MQ = KH // W1SPLIT         # m-chunks per w1 load chunk

BF16 = ml_dtypes.bfloat16

_nc_cache: dict = {}


def _plan_tiles(chunk_sizes, expert_ids):
    """Group each core's tokens by expert (tiles are just column sets), so
    tiles are uniform 512 wide regardless of chunk boundaries.

    Returns (perm, tiles): perm is the token order within a core's
    BPC*S-token slab (grouped by expert), tiles are (col0, T, e) in that
    permuted column space.
    """
    seg = []
    start = 0
    for sz, e in zip(chunk_sizes, expert_ids):
        seg.append((start, int(sz), int(e)))
        start += int(sz)
    expert_order = list(dict.fromkeys(e for _, _, e in seg))

    perm = []
    tiles = []
    col = 0
    for e in expert_order:
        for b in range(BPC):
            for (st, sz, ee) in seg:
                if ee == e:
                    perm.extend(range(b * S + st, b * S + st + sz))
        grp = len(perm) - col
        off = 0
        while off < grp:
            t = min(MAX_T, grp - off)
            tiles.append((col + off, t, e))
            off += t
        col += grp
    return tuple(perm), tuple(tiles)


def _build(tiles):
    import concourse.bacc as bacc
    import concourse.mybir as mybir
    import concourse.tile as tile

    dt = mybir.dt
    f32 = dt.float32
    bf16 = dt.bfloat16
    AF = mybir.ActivationFunctionType

    nc = bacc.Bacc("TRN2", target_bir_lowering=False, num_devices=NCORES)

    xb_d = nc.dram_tensor("xb", [P, KI * TPC], bf16, kind="ExternalInput")
    w1t_d = nc.dram_tensor("w1t", [E, W1SPLIT, P, KI * HID4], bf16,
                           kind="ExternalInput")
    w2t_d = nc.dram_tensor("w2t", [E, P, KH, OUT], bf16, kind="ExternalInput")
    bv_d = nc.dram_tensor("biasv", [P, KH + KO], f32, kind="ExternalInput")
    yb_d = nc.dram_tensor("yb", [P, KO * TPC], bf16, kind="ExternalOutput")

    # load weights in first-use order so tile 0 stalls only on its own
    # expert's first chunks, not on the full 18 MB weight set
    expert_order = list(dict.fromkeys([e for (_, _, e) in tiles]))
    expert_order += [e for e in range(E) if e not in expert_order]

    with tile.TileContext(nc) as tc:
        with (
            tc.tile_pool(name="const", bufs=1) as cpool,
            tc.tile_pool(name="w", bufs=1) as wpool,
            tc.tile_pool(name="xp", bufs=2) as xpool,
            tc.tile_pool(name="hp", bufs=KH + 2) as hpool,
            tc.tile_pool(name="yp", bufs=1) as ypool,
            tc.tile_pool(name="psh", bufs=2, space="PSUM") as psh,
            tc.tile_pool(name="psy", bufs=KO, space="PSUM") as psy,
        ):
            warm_s = cpool.tile([P, 1], f32)
            warmed = []
            bias_s = cpool.tile([P, KH + KO], f32)
            w1_s = [[None] * W1SPLIT for _ in range(E)]
            w2_s = [None] * E
            for i, e in enumerate(expert_order):
                for q in range(W1SPLIT):
                    w1_s[e][q] = wpool.tile([P, KI, HID4], bf16,
                                            name=f"w1_{e}_{q}")
                    nc.sync.dma_start(w1_s[e][q][:], w1t_d[e, q])
                    if i == 0 and q == 0:
                        # biases right after the first w1 chunk: off tile 0's
                        # critical path (q0) but well before the first gelu
                        nc.sync.dma_start(bias_s[:], bv_d.ap())
                w2_s[e] = wpool.tile([P, KH, OUT], bf16, name=f"w2_{e}")
                nc.sync.dma_start(w2_s[e][:], w2t_d[e])

            def do_tile(col0, T, e, y_s, yoff):
                """fc1 + gelu + fc2 + drains for one tile; y into y_s[yoff:]."""
                x_s = xpool.tile([P, KI * T], bf16, name="x", tag="x")
                # x/y ride the Activation HWDGE ring so they never queue
                # behind the weight stream on the SP ring; x loads in
                # k-halves so fc1 m=0 can start on k0-2 after half arrives
                xh = (KI // 2) * T
                nc.scalar.dma_start(
                    x_s[:, 0:xh], xb_d[:, KI * col0:KI * col0 + xh]
                )
                nc.scalar.dma_start(
                    x_s[:, xh:], xb_d[:, KI * col0 + xh:KI * (col0 + T)]
                )
                if not warmed:
                    # dummy 1-elem gelu AFTER tile 0's x DMAs on the Act SEQ:
                    # pulls the 1.28us act-table load into the startup DMA
                    # window without its FIFO dispatch delaying the x loads
                    warmed.append(True)
                    nc.vector.memset(warm_s[:], 0.0)
                    nc.scalar.activation(warm_s[:], warm_s[:], AF.Gelu)
                # fc1 + gelu: H^T chunks stay in SBUF
                hcs = []
                for m in range(KH):
                    h_ps = psh.tile([P, T], f32, name="hps", tag="hps")
                    q, mq = divmod(m, MQ)
                    for k in range(KI):
                        nc.tensor.matmul(
                            h_ps[:],
                            w1_s[e][q][:, k, mq * P:(mq + 1) * P],
                            x_s[:, k * T:(k + 1) * T],
                            start=(k == 0), stop=(k == KI - 1),
                        )
                    hc = hpool.tile([P, T], bf16, name="hc", tag="hc")
                    nc.scalar.activation(
                        hc[:], h_ps[:], AF.Gelu, bias=bias_s[:, m:m + 1]
                    )
                    hcs.append(hc)
                # fc2: per output chunk, contiguous accumulation per bank
                for o in range(KO):
                    y_ps = psy.tile([P, T], f32, name="yps", tag="yps")
                    for m in range(KH):
                        nc.tensor.matmul(
                            y_ps[:],
                            w2_s[e][:, m, o * P:(o + 1) * P],
                            hcs[m][:],
                            start=(m == 0), stop=(m == KH - 1),
                        )
                    nc.vector.tensor_scalar_add(
                        y_s[:, yoff + o * T:yoff + (o + 1) * T], y_ps[:],
                        bias_s[:, KH + o:KH + o + 1]
                    )

            # tiles in pairs with one merged y store per pair; the LAST pair
            # stores per tile (overlapping the penultimate store under the
            # final tile's compute) and the final tile's store is split in
            # halves so the first half transfers while the last drains run
            pairs = [tiles[i:i + 2] for i in range(0, len(tiles), 2)]
            for pi, pair in enumerate(pairs):
                if pi < len(pairs) - 1:
                    pcol0 = pair[0][0]
                    psum_t = sum(T for (_, T, _) in pair)
                    y_s = ypool.tile([P, KO * psum_t], bf16, name="y", tag="y")
                    yoff = 0
                    for (col0, T, e) in pair:
                        do_tile(col0, T, e, y_s, yoff)
                        yoff += KO * T
                    nc.scalar.dma_start(
                        yb_d[:, KO * pcol0:KO * pcol0 + KO * psum_t], y_s[:]
                    )
                else:
                    for ti, (col0, T, e) in enumerate(pair):
                        y_s = ypool.tile([P, KO * T], bf16, name="y", tag="y")
                        do_tile(col0, T, e, y_s, 0)
                        if ti == len(pair) - 1 and KO >= 6:
                            # final store in o-aligned pieces (3T, 2T, T) so
                            # the last piece's transfer is off the tail
                            c0 = KO * col0
                            for lo, hi in ((0, 3 * T), (3 * T, 5 * T),
                                           (5 * T, 6 * T)):
                                nc.scalar.dma_start(
                                    yb_d[:, c0 + lo:c0 + hi], y_s[:, lo:hi]
                                )
                        else:
                            nc.scalar.dma_start(
                                yb_d[:, KO * col0:KO * (col0 + T)], y_s[:]
                            )
    nc.compile()
    return nc


def _get_nc(tiles):
    nc = _nc_cache.get(tiles)
    if nc is None:
        nc = _nc_cache[tiles] = _build(tiles)
    return nc


def _prep_shared(inputs):
    """Merge LoRA into dense weights (f64), transpose, tile, cast bf16."""
    w1 = np.asarray(inputs["w1"], dtype=np.float64)
    b1 = np.asarray(inputs["b1"], dtype=np.float64)
    a1 = np.asarray(inputs["a1"], dtype=np.float64)
    w2 = np.asarray(inputs["w2"], dtype=np.float64)
    b2 = np.asarray(inputs["b2"], dtype=np.float64)
    a2 = np.asarray(inputs["a2"], dtype=np.float64)

    w1t = np.empty((E, W1SPLIT, P, KI * HID4), dtype=BF16)
    w2t = np.empty((E, P, KH, OUT), dtype=BF16)
    for e in range(E):
        w1e = (w1 + SCALE * (b1[e] @ a1[e])).T        # [IN, HID]
        w1f = w1e.reshape(KI, P, HID).transpose(1, 0, 2)   # [P, KI, HID]
        for q in range(W1SPLIT):
            w1t[e, q] = (
                w1f[:, :, q * HID4:(q + 1) * HID4]
                .reshape(P, KI * HID4).astype(BF16)
            )
        w2e = (w2 + SCALE * (b2[e] @ a2[e])).T        # [HID, OUT]
        w2t[e] = w2e.reshape(KH, P, OUT).transpose(1, 0, 2).astype(BF16)

    bias1 = np.asarray(inputs["bias1"], dtype=np.float32)
    bias2 = np.asarray(inputs["bias2"], dtype=np.float32)
    biasv = np.concatenate(
        [bias1.reshape(KH, P).T, bias2.reshape(KO, P).T], axis=1
    )                                                  # [P, KH + KO]
    return {
        "w1t": w1t,
        "w2t": w2t,
        "biasv": np.ascontiguousarray(biasv),
    }


def _make_in_maps(inputs, perm, tiles):
    x = np.asarray(inputs["x"], dtype=np.float32)
    shared = _prep_shared(inputs)
    xbf = x.astype(BF16).reshape(NCORES, TPC, IN)
    perm_a = np.asarray(perm)
    in_maps = []
    for c in range(NCORES):
        xp = xbf[c][perm_a]                          # [TPC, IN] permuted
        xb = np.empty((P, KI * TPC), dtype=BF16)
        for (col0, T, _e) in tiles:
            blk = xp[col0:col0 + T].reshape(T, KI, P).transpose(2, 1, 0)
            xb[:, KI * col0:KI * (col0 + T)] = blk.reshape(P, KI * T)
        m = dict(shared)
        m["xb"] = xb
        in_maps.append(m)
    return in_maps


def _assemble_y(results, perm, tiles):
    perm_a = np.asarray(perm)
    y = np.empty((NCORES, TPC, OUT), dtype=np.float32)
    yp = np.empty((TPC, OUT), dtype=np.float32)
    for c in range(NCORES):
        yb = results[c]["yb"]
        for (col0, T, _e) in tiles:
            t0 = KO * col0
            blk = yb[:, t0:t0 + KO * T].reshape(P, KO, T)
            yp[col0:col0 + T] = (
                blk.transpose(2, 1, 0).reshape(T, OUT).astype(np.float32)
            )
        y[c, perm_a] = yp
    return y.reshape(B, S, OUT)


def _run(inputs, trace=False):
    from concourse.bass_utils import run_bass_kernel_spmd

    chunk_sizes = tuple(int(v) for v in np.asarray(inputs["chunk_sizes"]))
    eids = tuple(int(v) for v in np.asarray(inputs["expert_indices"]))
    assert sum(chunk_sizes) == S

    perm, tiles = _plan_tiles(chunk_sizes, eids)
    nc = _get_nc(tiles)
    in_maps = _make_in_maps(inputs, perm, tiles)

    res = run_bass_kernel_spmd(
        nc, in_maps, core_ids=list(range(NCORES)), trace=trace
    )
    return _assemble_y(res.results, perm, tiles), res


def kernel(**inputs) -> np.ndarray:
    y, _ = _run(inputs, trace=False)
    return y
